# revision 11
# baseline (speedup 1.0000x reference)
"""EMA recurrent scan kernel for Trainium2 (Bass/Tile): hybrid DVE-scan +
PE Toeplitz-FIR, fp16 HBM I/O.

h_t = a*x_t + (1-a)*h_{t-1} over T=4096 for [B=8, D=1024, T] fp32;
B sharded across 8 cores; fp16 HBM I/O (host converts).

Per core the 1024 d-rows split into 8 partition-tiles of 128:
- k_scan tiles go through the baseline DVE tensor_tensor_scan path
  (natural [d, t] layout; 8.53 us/tile on DVE).
- The remaining (8-k) tiles go through the TensorE as a causal-Toeplitz
  FIR in TRANSPOSED layout (host stages x^T [T, D_pe]):
    out^T[t_out, bd] = sum_k WL[k, t_out] * x^T[c*128+k, bd]   (own chunk)
                     + sum_k WU[k, t_out] * x^T[(c-1)*128+k, bd] (prev chunk)
  with WL[k,m] = a*b^(m-k) (k<=m), WU[k,m] = a*b^(m+128-k), b = 1-a.
  b^128 ~ 1e-28 so two chunks of history are exact; fp16 underflow
  truncates coefficients below ~6e-8 (error ~1e-7 relative).
  h0 enters chunk 0 as a rank-1 K=1 matmul with v[m] = b^(m+1).
  PSUM accumulates fp32; ACT/DVE evacuate to fp16; host re-transposes.
"""

import numpy as np

import concourse.bass as bass
import concourse.mybir as mybir
from concourse import bass_utils, tile

ALPHA = 0.4
B, D, T = 8, 1024, 4096
N_CORES = 8
P = 128
N_TILES = D // P  # 8
CHUNKS = T // P  # 32

IO_DT = mybir.dt.float16
IO_NP = np.float16

K_SCAN = 2  # tiles on the DVE path; 8-K_SCAN on the PE path


def _split_excess_waits(nc: bass.Bass) -> None:
    """Walrus allows one sync-wait slot per instruction: hoist extras onto
    same-engine NoOps immediately before (identical blocking semantics)."""
    for f in nc.m.functions:
        for blk in f.blocks:
            new_insts = []
            changed = False
            for inst in blk.instructions:
                si = inst.sync_info
                if si is not None and si.on_wait and len(si.on_wait) > 1:
                    waits = list(si.on_wait)
                    for kk, w in enumerate(waits[:-1]):
                        new_insts.append(
                            mybir.InstNoOp(
                                name=f"{inst.name}.w{kk}",
                                engine=inst.engine,
                                sync_info=mybir.SyncInfo(on_wait=[w], on_update=[]),
                                bass_nofuse=True,
                            )
                        )
                    inst.sync_info = mybir.SyncInfo(
                        on_wait=[waits[-1]], on_update=list(si.on_update)
                    )
                    changed = True
                new_insts.append(inst)
            if changed:
                blk.instructions = new_insts


def _w_const() -> np.ndarray:
    """[128, 384] fp16: cols 0:128 WL, 128:256 WU, row 0 of 256:384 = v."""
    a = abs(ALPHA)
    b = 1.0 - a
    k = np.arange(P)[:, None]
    m = np.arange(P)[None, :]
    wl = np.where(k <= m, a * b ** (m - k), 0.0)
    wu = a * b ** ((m + P) - k)
    v = b ** (np.arange(P) + 1.0)
    w = np.zeros((P, 3 * P), np.float32)
    w[:, 0:P] = wl
    w[:, P : 2 * P] = wu
    w[0, 2 * P : 3 * P] = v
    return w.astype(IO_NP)


def _build_nc(reps: int = 1, k_scan: int = K_SCAN, evac3: bool = False,
              wbatch: bool = True, half_store: bool = False,
              store_split: bool = True) -> bass.Bass:
    a = abs(ALPHA)
    n_pe = N_TILES - k_scan
    d_pe = n_pe * P
    groups = []  # (col0, width) bd-groups of <=512 for the PE path
    c0 = 0
    while c0 < d_pe:
        w = min(512, d_pe - c0)
        groups.append((c0, w))
        c0 += w

    nc = bass.Bass(trn_type="TRN2")
    if k_scan:
        xn = nc.dram_tensor("inp_nat", [k_scan * P, T], IO_DT, kind="ExternalInput")
        h0n = nc.dram_tensor("h0n", [k_scan * P, 1], mybir.dt.float32, kind="ExternalInput")
        yn = nc.dram_tensor("out_nat", [k_scan * P, T], IO_DT, kind="ExternalOutput")
    if n_pe:
        xt = nc.dram_tensor("inp_tr", [T, d_pe], IO_DT, kind="ExternalInput")
        h0t = nc.dram_tensor("h0t", [1, d_pe], IO_DT, kind="ExternalInput")
        wc = nc.dram_tensor("wconst", [P, 3 * P], IO_DT, kind="ExternalInput")
        yt = nc.dram_tensor("out_tr", [T, d_pe], IO_DT, kind="ExternalOutput")

    with tile.TileContext(nc) as tc:
        with (
            tc.tile_pool(name="const", bufs=1) as cpool,
            tc.tile_pool(name="io", bufs=2) as pool,
            tc.psum_pool(name="acc", bufs=8) as ppool,
        ):
            # ---- constants ----
            if k_scan:
                decay = cpool.tile([P, T], mybir.dt.float32)
                nc.vector.memset(decay[:, :], 1.0 - a)
                h0_all = cpool.tile([P, k_scan], mybir.dt.float32)
                nc.sync.dma_start(
                    h0_all[:, :], h0n.rearrange("(t p) o -> p (t o)", p=P)
                )
            if n_pe:
                wcs = cpool.tile([P, 3 * P], IO_DT)
                nc.sync.dma_start(wcs[:, :], wc[:, :])
                wl = wcs[:, 0:P]
                wu = wcs[:, P : 2 * P]
                wh = wcs[0:1, 2 * P : 3 * P]
                h0sb = cpool.tile([1, d_pe], IO_DT)
                nc.sync.dma_start(h0sb[:, :], h0t[:, :])

            evacs = [
                lambda o, i: nc.scalar.mul(o, i, 1.0),
                lambda o, i: nc.vector.tensor_copy(o, i),
                lambda o, i: nc.gpsimd.tensor_copy(o, i),
            ]
            n_ev = 3 if evac3 else 2

            def emit_scan_tile(i: int):
                """One DVE-scan tile (d-rows i*128..), full T. The host
                pre-scales inp_nat by a, so the scan consumes it directly."""
                xg = pool.tile([P, T], IO_DT, tag="sx", name="sx", bufs=3)
                nc.sync.dma_start(xg[:, :], xn[i * P : (i + 1) * P, :])
                sg = pool.tile([P, T], IO_DT, tag="ss", name="ss", bufs=3)
                nc.vector.tensor_tensor_scan(
                    sg[:, :], decay[:, :], xg[:, :], h0_all[:, i : i + 1],
                    op0=mybir.AluOpType.mult, op1=mybir.AluOpType.add,
                )
                st_eng = nc.scalar if store_split else nc.gpsimd
                st_eng.dma_start(yn[i * P : (i + 1) * P, :], sg[:, :])

            def emit_pe_super(s: int, ss: int, prev_tile, ev_idx: int):
                """Load ss x^T chunks in ONE 1 MiB DMA, run the L/U matmul
                pairs per chunk x bd-group, evac to a [P, ss, d_pe] out
                super-tile, store it with one DMA."""
                c0 = s * ss
                ct = pool.tile([P, ss, d_pe], IO_DT, tag="px", name="px", bufs=3)
                nc.sync.dma_start(
                    ct[:, :, :],
                    xt[c0 * P : (c0 + ss) * P, :].rearrange(
                        "(j p) d -> p j d", p=P
                    ),
                )
                ot = pool.tile([P, ss, d_pe], IO_DT, tag="po", name="po", bufs=3)
                ev = ev_idx

                def rprev_of(j):
                    return ct[:, j - 1, :] if j > 0 else (
                        prev_tile[:, ss - 1, :] if prev_tile is not None else None
                    )

                if wbatch:
                    # batch by stationary: all L MMs, then all U MMs
                    pss = {}
                    for j in range(ss):
                        for gi, (g0, gw) in enumerate(groups):
                            ps = pss[(j, gi)] = ppool.tile(
                                [P, 512], mybir.dt.float32, tag="ps", name="ps", bufs=8
                            )
                            nc.tensor.matmul(
                                ps[:, :gw], wl, ct[:, j, g0 : g0 + gw],
                                start=True, stop=False,
                            )
                    for j in range(ss):
                        c = c0 + j
                        rprev = rprev_of(j)
                        for gi, (g0, gw) in enumerate(groups):
                            ps = pss[(j, gi)]
                            if c == 0:
                                nc.tensor.matmul(
                                    ps[:, :gw], wh, h0sb[0:1, g0 : g0 + gw],
                                    start=False, stop=True,
                                )
                            else:
                                nc.tensor.matmul(
                                    ps[:, :gw], wu, rprev[:, g0 : g0 + gw],
                                    start=False, stop=True,
                                )
                            evacs[ev % n_ev](ot[:, j, g0 : g0 + gw], ps[:, :gw])
                            ev += 1
                        if half_store and j == ss // 2 - 1:
                            # first half-store right after its evacs: the
                            # store stream overlaps the remaining U/evac work
                            nc.gpsimd.dma_start(
                                yt[c0 * P : (c0 + ss // 2) * P, :].rearrange(
                                    "(j p) d -> p j d", p=P
                                ),
                                ot[:, : ss // 2, :],
                            )
                    if half_store:
                        nc.gpsimd.dma_start(
                            yt[(c0 + ss // 2) * P : (c0 + ss) * P, :].rearrange(
                                "(j p) d -> p j d", p=P
                            ),
                            ot[:, ss // 2 :, :],
                        )
                        return ct
                else:
                    for j in range(ss):
                        c = c0 + j
                        rprev = rprev_of(j)
                        for gi, (g0, gw) in enumerate(groups):
                            ps = ppool.tile(
                                [P, 512], mybir.dt.float32, tag="ps", name="ps", bufs=8
                            )
                            nc.tensor.matmul(
                                ps[:, :gw], wl, ct[:, j, g0 : g0 + gw],
                                start=True, stop=False,
                            )
                            if c == 0:
                                nc.tensor.matmul(
                                    ps[:, :gw], wh, h0sb[0:1, g0 : g0 + gw],
                                    start=False, stop=True,
                                )
                            else:
                                nc.tensor.matmul(
                                    ps[:, :gw], wu, rprev[:, g0 : g0 + gw],
                                    start=False, stop=True,
                                )
                            evacs[ev % n_ev](ot[:, j, g0 : g0 + gw], ps[:, :gw])
                            ev += 1
                st_eng = (nc.scalar if s % 2 else nc.gpsimd) if store_split else nc.gpsimd
                st_eng.dma_start(
                    yt[c0 * P : (c0 + ss) * P, :].rearrange(
                        "(j p) d -> p j d", p=P
                    ),
                    ot[:, :, :],
                )
                return ct

            def body():
                prev = None
                ev = 0
                ss = 4
                if n_pe:
                    next_scan = 0
                    n_super = CHUNKS // ss
                    for s in range(n_super):
                        while next_scan < k_scan and s == (next_scan * n_super) // k_scan:
                            emit_scan_tile(next_scan)
                            next_scan += 1
                        prev = emit_pe_super(s, ss, prev, ev)
                        ev += ss * len(groups)
                    while next_scan < k_scan:
                        emit_scan_tile(next_scan)
                        next_scan += 1
                else:
                    for i in range(k_scan):
                        emit_scan_tile(i)

            for _ in range(reps):
                body()

    _split_excess_waits(nc)
    return nc


def _in_maps(inp: np.ndarray, hidden: np.ndarray, k_scan: int = K_SCAN):
    inp = np.asarray(inp)
    hidden = np.ascontiguousarray(np.asarray(hidden, dtype=np.float32))
    assert inp.shape == (B, D, T), inp.shape
    wc = _w_const()
    ds = k_scan * P
    maps = []
    for b in range(N_CORES):
        m = {}
        if k_scan:
            # host-side a-prescale: the DVE scan consumes a*x directly
            m["inp_nat"] = np.ascontiguousarray(
                (abs(ALPHA) * inp[b, :ds]).astype(IO_NP)
            )
            m["h0n"] = np.ascontiguousarray(hidden[b, :ds])
        if k_scan < N_TILES:
            m["inp_tr"] = np.ascontiguousarray(inp[b, ds:].T.astype(IO_NP))
            m["h0t"] = np.ascontiguousarray(hidden[b, ds:, 0][None, :].astype(IO_NP))
            m["wconst"] = wc
        maps.append(m)
    return maps


def _assemble(results, k_scan: int = K_SCAN) -> np.ndarray:
    out = np.empty((B, D, T), np.float32)
    ds = k_scan * P
    for b in range(N_CORES):
        if k_scan:
            out[b, :ds] = results[b]["out_nat"].astype(np.float32)
        if k_scan < N_TILES:
            out[b, ds:] = results[b]["out_tr"].T.astype(np.float32)
    return out


_NC_CACHE: bass.Bass | None = None


def _get_nc() -> bass.Bass:
    global _NC_CACHE
    if _NC_CACHE is None:
        _NC_CACHE = _build_nc()
    return _NC_CACHE


def _run(inp: np.ndarray, hidden: np.ndarray, nc: bass.Bass | None = None,
         k_scan: int = K_SCAN, **spmd_kwargs):
    in_maps = _in_maps(inp, hidden, k_scan)
    res = bass_utils.run_bass_kernel_spmd(
        nc if nc is not None else (
            _get_nc() if k_scan == K_SCAN else _build_nc(k_scan=k_scan)
        ),
        in_maps,
        core_ids=list(range(N_CORES)),
        **spmd_kwargs,
    )
    return _assemble(res.results, k_scan), res


def kernel(inp: np.ndarray, hidden: np.ndarray) -> np.ndarray:
    out, _ = _run(inp, hidden)
    return out
